# revision 1
# baseline (speedup 1.0000x reference)
"""Sliding-window causal attention (B=1, S=4096, E=1024, H=16, D=64,
window=(256,0)) on 8 TRN2 NeuronCores.

Sharding: pure sequence-parallel. Core c computes queries [512c, 512c+512)
and needs keys [512c-256, 512c+512) -- a 256-row halo. No collectives.

Layout: everything transposed ([e, s]) so QKV projections, the RoPE
rotation, scores, PV and the output projection are all TensorE matmuls.
Matmul inputs are bf16; accumulation is f32 in PSUM.

v2 restructure (trace-driven):
 - Unified software pipeline: one loop over the 8 embed tiles that
   interleaves QK projection + RoPE of tile et with scores of head pair
   et-1 and PV of head pair et-2 (V projection interleaved in the first
   two iterations).  Dense projection matmuls pad the PE while ACT/DVE
   chew on exp/mask, so the HAM clock gate stays warm (the v1 trace ran
   the whole attention phase at 1.2 GHz).
 - Scores for one head are bank-packed: 8 matmuls fill 3 PSUM banks
   exactly (no garbage), so exp is 3 big ACT ops per head instead of 12
   small ones, and the window mask is ONE batched DVE multiply per head
   against a static packed pattern.
 - DMA order: wq/x tiles first (they gate the first matmul), cos/sin/
   masks later; warm-up matmuls on a memset tile warm the PE clock
   during the load window.
"""

import os
import sys

sys.path.insert(0, "/opt/trn_rl_repo")

import math

import numpy as np
import ml_dtypes

import concourse.bass as bass
from concourse import bacc
import concourse.mybir as mybir
import concourse.tile as tile
from concourse.bass_utils import run_bass_kernel_spmd


def _ensure_ntff_hook():
    """Register the axon NTFF profile hook so trace=True works."""
    import types
    try:
        import antenv
    except ImportError:
        return
    if "antenv.axon_hooks" not in sys.modules:
        mod = types.ModuleType("antenv.axon_hooks")
        mod._hook = None
        def _set(h, _m=mod):
            _m._hook = h
        def _get(_m=mod):
            return _m._hook
        mod.set_axon_ntff_profile_hook = _set
        mod.get_axon_ntff_profile_hook = _get
        sys.modules["antenv.axon_hooks"] = mod
        antenv.axon_hooks = mod
    hooks = sys.modules["antenv.axon_hooks"]
    if hooks.get_axon_ntff_profile_hook() is None:
        try:
            from trn_agent_boot.trn_boot import _ntff_profile_via_ctypes
            hooks.set_axon_ntff_profile_hook(
                _ntff_profile_via_ctypes("/opt/axon/libaxon_pjrt.so"))
        except Exception:
            pass

BF16 = mybir.dt.bfloat16
F32 = mybir.dt.float32

NCORES = 8
S = 4096
E = 1024
H = 16
D = 64
SL = S // NCORES        # 512 local queries per core
HALO = 256
SK = SL + HALO          # 768 local keys (padded frame)
NQT = SL // 128         # 4 query tiles
NKT = SK // 128         # 6 key tiles
NET = E // 128          # 8 embed tiles
SCALE = 1.0 / math.sqrt(D)
NWARM = 20              # PE warm-up matmuls during the DMA load window

# Bank-packed score layout for one head: the 12 (kt, qtile) band blocks
# (128 cols each) fill 3 PSUM banks of 512 f32 exactly.  Each entry is
# one matmul: (bank, bank_col, kt, qlo, qhi).
SCORE_BLOCKS = [
    (0, 0,   0, 0,   128),
    (0, 128, 1, 0,   256),
    (0, 384, 2, 0,   128),
    (1, 0,   2, 128, 384),
    (1, 256, 3, 128, 384),
    (2, 0,   3, 384, 512),
    (2, 128, 4, 256, 512),
    (2, 384, 5, 384, 512),
]
# (kt, qtile) -> column offset of that 128-col block in the packed
# [128, 1536] probability tile.
PV_OFF = {}
for _b, _c, _kt, _qlo, _qhi in SCORE_BLOCKS:
    for _i in range((_qhi - _qlo) // 128):
        PV_OFF[(_kt, _qlo // 128 + _i)] = _b * 512 + _c + _i * 128


def _build_graph():
    nc = bacc.Bacc("TRN2", target_bir_lowering=False, debug=False, num_devices=NCORES)

    # ---- DRAM parameters (per-core shards staged by kernel()) ----
    xT = nc.declare_dram_parameter("xT", [E, SK], BF16, isOutput=False)
    wq = nc.declare_dram_parameter("wq", [E, E], BF16, isOutput=False)
    wk = nc.declare_dram_parameter("wk", [E, E], BF16, isOutput=False)
    wv = nc.declare_dram_parameter("wv", [E, E], BF16, isOutput=False)
    wo = nc.declare_dram_parameter("wo", [E, E], BF16, isOutput=False)
    rt = nc.declare_dram_parameter("rt", [128, 128], BF16, isOutput=False)
    bq = nc.declare_dram_parameter("bq", [128, NET], F32, isOutput=False)
    bo = nc.declare_dram_parameter("bo", [128, NET], F32, isOutput=False)
    bv = nc.declare_dram_parameter("bv", [128, E], F32, isOutput=False)
    cosT = nc.declare_dram_parameter("cosT", [128, SK], BF16, isOutput=False)
    sinT = nc.declare_dram_parameter("sinT", [128, SK], BF16, isOutput=False)
    maskpat = nc.declare_dram_parameter("maskpat", [128, 1536], BF16, isOutput=False)
    # bf16 output: halves the tail DMA; host upcasts to f32 (the ~0.2%
    # quantization is well inside the error budget)
    out_ext = nc.declare_dram_parameter("out", [E, SL], BF16, isOutput=True)

    with tile.TileContext(nc) as tc:
        with (
            tc.tile_pool(name="wpool", bufs=1) as wpool,
            tc.tile_pool(name="xpool", bufs=1) as xpool,
            tc.tile_pool(name="qk", bufs=1) as qkpool,
            tc.tile_pool(name="vpool", bufs=1) as vpool,
            tc.tile_pool(name="cs", bufs=1) as cspool,
            tc.tile_pool(name="small", bufs=1) as small,
            tc.tile_pool(name="rope", bufs=2) as ropepool,
            tc.tile_pool(name="pe", bufs=2) as pepool,
            tc.tile_pool(name="pm", bufs=4) as pmpool,
            tc.tile_pool(name="att", bufs=2) as attpool,
            tc.tile_pool(name="ctx", bufs=1) as ctxpool,
            tc.tile_pool(name="outp", bufs=2) as outpool,
            tc.tile_pool(name="mm", bufs=3, space="PSUM") as mmps,
            tc.tile_pool(name="sc", bufs=3, space="PSUM") as scps,
            tc.tile_pool(name="cx", bufs=2, space="PSUM") as cxps,
        ):
            # ---------- loads, ordered by who needs them first ----------
            warm_sb = small.tile([128, 512], BF16, tag="warm")
            nc.vector.memset(warm_sb[:], 0.0)
            ones_sb = small.tile([1, 64], F32, tag="ones")
            nc.vector.memset(ones_sb[:], 1.0)

            rt_sb = small.tile([128, 128], BF16, tag="rt")
            nc.sync.dma_start(rt_sb[:], rt[:])
            cos_sb = cspool.tile([128, SK], BF16, tag="cos")
            nc.sync.dma_start(cos_sb[:], cosT[:])
            sin_sb = cspool.tile([128, SK], BF16, tag="sin")
            nc.sync.dma_start(sin_sb[:], sinT[:])
            bq_sb = small.tile([128, NET], F32, tag="bq")
            nc.sync.dma_start(bq_sb[:], bq[:])

            # wq/x/wk tiles interleaved: they pace the iteration-0
            # projection matmuls; wv follows (its matmuls are interleaved
            # into iterations 1-2 of the in-order PE queue).
            x_sb = []
            w_sb = {"q": [], "k": [], "v": []}
            for kt in range(NET):
                t = wpool.tile([128, E], BF16, tag=f"wq{kt}")
                nc.sync.dma_start(t[:], wq[kt * 128 : (kt + 1) * 128, :])
                w_sb["q"].append(t)
                t = xpool.tile([128, SK], BF16, tag=f"x{kt}")
                nc.sync.dma_start(t[:], xT[kt * 128 : (kt + 1) * 128, :])
                x_sb.append(t)
                t = wpool.tile([128, E], BF16, tag=f"wk{kt}")
                nc.sync.dma_start(t[:], wk[kt * 128 : (kt + 1) * 128, :])
                w_sb["k"].append(t)
            for kt in range(NET):
                t = wpool.tile([128, E], BF16, tag=f"wv{kt}")
                nc.sync.dma_start(t[:], wv[kt * 128 : (kt + 1) * 128, :])
                w_sb["v"].append(t)
            bv_sb = small.tile([128, E], F32, tag="bv")
            nc.sync.dma_start(bv_sb[:], bv[:])
            mask_sb = cspool.tile([128, 1536], BF16, tag="mask")
            nc.sync.dma_start(mask_sb[:], maskpat[:])
            wo_sb = []
            for kt in range(NET):
                t = wpool.tile([128, E], BF16, tag=f"wo{kt}")
                nc.sync.dma_start(t[:], wo[kt * 128 : (kt + 1) * 128, :])
                wo_sb.append(t)
            bo_sb = small.tile([128, NET], F32, tag="bo")
            nc.sync.dma_start(bo_sb[:], bo[:])

            # Warm-up matmuls: serial WAW chain through the mm pool keeps
            # the PE busy through the HAM window while weights stream in.
            warm_ctr = [0]

            def warm_mm():
                i = warm_ctr[0]
                warm_ctr[0] += 1
                wp = mmps.tile([128, 512], F32, tag="mm", name=f"warm{i}")
                nc.tensor.matmul(wp[:], warm_sb[:, 0:128], warm_sb[:],
                                 start=True, stop=True)

            for i in range(NWARM):
                warm_mm()

            # ---------- pipelined main loop ----------
            q_rope = [None] * NET   # [128, SL] bf16 (rows = 2 heads x 64)
            k_rope = [None] * NET   # [128, SK] bf16
            v_sb = [None] * NKT     # [128, 16*65] bf16 (rows = local seq)
            ctx_sb = [ctxpool.tile([128, SL], BF16, tag=f"ctx{et}",
                                   name=f"ctx{et}")
                      for et in range(NET)]
            pm_tiles = {}

            def project(which, et, n0, n1, psum, trickle=False):
                for kt in range(NET):
                    nc.tensor.matmul(
                        psum[:, 0 : n1 - n0],
                        w_sb[which][kt][:, et * 128 : (et + 1) * 128],
                        x_sb[kt][:, n0:n1],
                        start=(kt == 0),
                        stop=(kt == NET - 1),
                    )
                    if trickle and kt % 2 == 1 and kt < NET - 1:
                        # DMA-paced matmuls leave PE idle gaps that cool
                        # the HAM clock gate; independent warm matmuls
                        # between them keep it at 8/8.
                        warm_mm()

            def emit_qk_rope(et):
                trick = et == 0
                # q: only real rows (cols HALO..SK of padded frame)
                qp = mmps.tile([128, 512], F32, tag="mm")
                project("q", et, HALO, SK, qp, trickle=trick)
                q_lin = ropepool.tile([128, SL], BF16, tag="qlin")
                nc.scalar.activation(
                    q_lin[:], qp[:, 0:SL],
                    mybir.ActivationFunctionType.Identity,
                    bias=bq_sb[:, et : et + 1], scale=1.0,
                )
                # k: all SK rows, no bias
                kp = mmps.tile([128, 512], F32, tag="mm")
                project("k", et, 0, 512, kp, trickle=trick)
                kp2 = mmps.tile([128, 512], F32, tag="mm")
                project("k", et, 512, SK, kp2, trickle=trick)
                k_lin = ropepool.tile([128, SK], BF16, tag="klin")
                nc.scalar.copy(k_lin[:, 0:512], kp[:, 0:512])
                nc.scalar.copy(k_lin[:, 512:SK], kp2[:, 0 : SK - 512])
                # rotate_half via matmul with the static rotation matrix
                rotp = mmps.tile([128, 512], F32, tag="mm", name="rotp")
                nc.tensor.matmul(rotp[:, 0:SL], rt_sb[:], q_lin[:],
                                 start=True, stop=True)
                rotk = mmps.tile([128, 512], F32, tag="mm", name="rotk")
                nc.tensor.matmul(rotk[:, :], rt_sb[:], k_lin[:, 0:512],
                                 start=True, stop=True)
                rotk2 = mmps.tile([128, 512], F32, tag="mm", name="rotk2")
                nc.tensor.matmul(rotk2[:, 0 : SK - 512], rt_sb[:],
                                 k_lin[:, 512:SK], start=True, stop=True)
                # all-bf16 SBUF temporaries hit the DVE 2x/4x perf modes
                t1 = ropepool.tile([128, SL], BF16, tag="t1", name="t1")
                nc.vector.tensor_mul(t1[:], q_lin[:], cos_sb[:, HALO:SK])
                t2 = ropepool.tile([128, SL], BF16, tag="t2", name="t2")
                nc.vector.tensor_mul(t2[:], rotp[:, 0:SL], sin_sb[:, HALO:SK])
                qf = qkpool.tile([128, SL], BF16, tag=f"qf{et}", name=f"qf{et}")
                nc.vector.tensor_add(qf[:], t1[:], t2[:])
                q_rope[et] = qf
                t3 = ropepool.tile([128, SK], BF16, tag="t3", name="t3")
                nc.vector.tensor_mul(t3[:], k_lin[:], cos_sb[:])
                t4 = ropepool.tile([128, SK], BF16, tag="t4", name="t4")
                nc.vector.tensor_mul(t4[:, 0:512], rotk[:, :], sin_sb[:, 0:512])
                nc.vector.tensor_mul(t4[:, 512:SK], rotk2[:, 0 : SK - 512],
                                     sin_sb[:, 512:SK])
                kf = qkpool.tile([128, SK], BF16, tag=f"kf{et}", name=f"kf{et}")
                nc.vector.tensor_add(kf[:], t3[:], t4[:])
                k_rope[et] = kf

            def emit_v(st):
                # natural layout [s, e]; per head 64 value dims + 1 ones
                # column so PV's 65th output row collects sum(P) for free.
                vt = vpool.tile([128, 16 * 65], BF16, tag=f"v{st}")
                nc.gpsimd.memset(
                    vt[:].rearrange("p (h c) -> p h c", c=65)[:, :, 64:65], 1.0
                )
                for half in range(2):
                    # cx pool is idle until the first PV: using it here
                    # keeps the mm pool free for the qk-projection flow.
                    vp = cxps.tile([128, 512], F32, tag="cx")
                    for kt in range(NET):
                        nc.tensor.matmul(
                            vp[:],
                            x_sb[kt][:, st * 128 : (st + 1) * 128],
                            w_sb["v"][kt][:, half * 512 : (half + 1) * 512],
                            start=(kt == 0),
                            stop=(kt == NET - 1),
                        )
                        if kt == 3:
                            warm_mm()  # wv is still streaming in
                    dst = vt[:, half * 8 * 65 : (half * 8 + 8) * 65].rearrange(
                        "p (h c) -> p h c", c=65
                    )[:, :, 0:64]
                    nc.vector.tensor_add(
                        dst, vp[:].rearrange("p (h c) -> p h c", c=64),
                        bv_sb[:, half * 512 : (half + 1) * 512].rearrange(
                            "p (h c) -> p h c", c=64
                        ),
                    )
                v_sb[st] = vt

            def emit_scores(hp, sub):
                # one head: 8 matmuls bank-pack the 12 band blocks into 3
                # PSUM banks; 3 big exps; one batched mask multiply.
                et = hp
                banks = [scps.tile([128, 512], F32, tag="sc",
                                   name=f"sc{hp}_{sub}_{b}") for b in range(3)]
                for (b, c, kt, qlo, qhi) in SCORE_BLOCKS:
                    nc.tensor.matmul(
                        banks[b][:, c : c + (qhi - qlo)],
                        k_rope[et][sub : sub + 64, kt * 128 : (kt + 1) * 128],
                        q_rope[et][sub : sub + 64, qlo:qhi],
                        start=True, stop=True,
                    )
                pe = pepool.tile([128, 1536], BF16, tag="pe",
                                 name=f"pe{hp}_{sub}")
                for b in range(3):
                    nc.scalar.activation(
                        pe[:, b * 512 : (b + 1) * 512], banks[b][:],
                        mybir.ActivationFunctionType.Exp,
                        bias=0.0, scale=SCALE,
                    )
                pm = pmpool.tile([128, 1536], BF16, tag="pm",
                                 name=f"pm{hp}_{sub}")
                # (measured: GpSimd for this op is ~5x slower and stalls
                # PV -- keep it on DVE)
                nc.vector.tensor_mul(pm[:], pe[:], mask_sb[:])
                pm_tiles[(hp, sub)] = pm

            def emit_pv(hp, sub, pe_bcast=False):
                et = hp
                h = 2 * hp + (sub // 64)
                pm = pm_tiles.pop((hp, sub))
                cxp = cxps.tile([128, 512], F32, tag="cx")
                # One matmul per k-tile: its band blocks are contiguous in
                # the packed pm AND query-aligned, so each matmul covers
                # the kt's whole query range.  PSUM per-element
                # has_written bits make the staggered accumulation exact:
                # the first matmul touching a column overwrites, later
                # ones accumulate.
                for kt in range(NKT):
                    lo = max(0, kt - 2) * 128
                    hi = min(kt + 1, NQT) * 128
                    off = PV_OFF[(kt, lo // 128)]
                    nc.tensor.matmul(
                        cxp[0:65, lo:hi],
                        v_sb[kt][:, h * 65 : (h + 1) * 65],
                        pm[:, off : off + (hi - lo)],
                        start=(kt == 0), stop=(kt == NKT - 1),
                    )
                # tensor_copy can shift partitions (64 -> 0); the custom
                # reciprocal op cannot, so keep the copy before it.
                lr = attpool.tile([1, SL], F32, tag="lr")
                nc.vector.tensor_copy(lr[:], cxp[64:65, :])
                linv = attpool.tile([1, SL], F32, tag="linv")
                nc.vector.reciprocal_approx_fast(linv[:], lr[:])
                if pe_bcast:
                    # tail head pair: out-proj waits on this chain, so
                    # broadcast 1/l on the (idle) PE instead of GpSimd.
                    # DVE can read only one PSUM operand, so ACT (also
                    # idle) evacuates ctx in parallel with the recip.
                    craw = attpool.tile([64, SL], F32, tag="craw")
                    nc.scalar.copy(craw[:], cxp[0:64, :])
                    lbc_ps = mmps.tile([128, 512], F32, tag="mm",
                                       name=f"lbc{h}")
                    nc.tensor.matmul(lbc_ps[0:64, :], ones_sb[:],
                                     linv[:], start=True, stop=True)
                    nc.vector.tensor_mul(ctx_sb[et][sub : sub + 64, :],
                                         craw[:], lbc_ps[0:64, :])
                else:
                    lbc_sb = attpool.tile([64, SL], F32, tag="lbc")
                    nc.gpsimd.partition_broadcast(lbc_sb[:], linv[:])
                    nc.vector.tensor_mul(ctx_sb[et][sub : sub + 64, :],
                                         cxp[0:64, :], lbc_sb[:])

            for et in range(NET):
                # The last rope is pulled one iteration forward: its
                # serial DVE chain then overlaps iteration 7's attention
                # matmuls instead of gating the drain (measured 3.3us
                # PE gap at loop exit).
                if et < NET - 1:
                    emit_qk_rope(et)
                    if et == NET - 2:
                        emit_qk_rope(NET - 1)
                if et == 1:
                    # V matmuls double as gap filler while ACT exps head
                    # pair 0; wv lands mid-iteration.
                    emit_scores(0, 0)
                    emit_v(0)
                    emit_v(1)
                    emit_v(2)
                    emit_scores(0, 64)
                elif et == 2:
                    emit_scores(1, 0)
                    emit_v(3)
                    emit_v(4)
                    emit_v(5)
                    emit_pv(0, 0)
                    emit_pv(0, 64)
                    emit_scores(1, 64)
                elif et >= 3:
                    # both PV fills between the two score blocks: the
                    # second head's score matmuls reuse the sc banks, so
                    # they wait on the first head's exps -- give them
                    # ~2.8us of independent PV work to hide behind.
                    emit_scores(et - 1, 0)
                    emit_pv(et - 2, 0)
                    emit_pv(et - 2, 64)
                    if et == NET - 1:
                        # last iteration has no projection matmuls left
                        # to hide the exp waits (measured 2.3us+1.1us
                        # gaps) -- warm matmuls are free here
                        for _ in range(5):
                            warm_mm()
                    emit_scores(et - 1, 64)
                    if et == NET - 1:
                        warm_mm()
                        warm_mm()
            # drain the pipeline
            # Drain: pv(6,0) leads -- its inputs are ready at loop exit
            # and it fills the wait for head pair 6's last exps (in the
            # steady loop the next iteration's projections hid that).
            # Warm matmuls plug the remaining chain slivers so the HAM
            # clock stays at 8/8 through the out-projection.
            emit_pv(NET - 2, 0, pe_bcast=True)
            emit_scores(NET - 1, 0)
            emit_pv(NET - 2, 64, pe_bcast=True)
            warm_mm()
            emit_scores(NET - 1, 64)
            warm_mm()
            emit_pv(NET - 1, 0, pe_bcast=True)
            warm_mm()
            emit_pv(NET - 1, 64, pe_bcast=True)
            warm_mm()

            # ---------- output projection ----------
            def finish_out(eo, op):
                o_sb = outpool.tile([128, SL], BF16, tag="o")
                nc.scalar.activation(
                    o_sb[:], op[:], mybir.ActivationFunctionType.Identity,
                    bias=bo_sb[:, eo : eo + 1], scale=1.0,
                )
                nc.sync.dma_start(out_ext[eo * 128 : (eo + 1) * 128, :], o_sb[:])

            # Partial accumulation over et 0..6 for three output tiles:
            # these matmuls fill the PE while the last head pair's
            # exp/mask/normalize chains run, instead of gapping cold.
            # (sc-pool banks are free once the last scores are exp'd.)
            op_part = []
            for eo in range(3):
                op = scps.tile([128, 512], F32, tag="sc", name=f"op{eo}")
                for et in range(NET - 1):
                    nc.tensor.matmul(
                        op[:],
                        wo_sb[et][:, eo * 128 : (eo + 1) * 128],
                        ctx_sb[et][:],
                        start=(et == 0),
                        stop=False,
                    )
                op_part.append(op)
            warm_mm()
            for eo in range(3):
                nc.tensor.matmul(
                    op_part[eo][:],
                    wo_sb[NET - 1][:, eo * 128 : (eo + 1) * 128],
                    ctx_sb[NET - 1][:],
                    start=False,
                    stop=True,
                )
                finish_out(eo, op_part[eo])
            for eo in range(3, NET):
                op = mmps.tile([128, 512], F32, tag="mm")
                for et in range(NET):
                    nc.tensor.matmul(
                        op[:],
                        wo_sb[et][:, eo * 128 : (eo + 1) * 128],
                        ctx_sb[et][:],
                        start=(et == 0),
                        stop=(et == NET - 1),
                    )
                finish_out(eo, op)

    nc.compile()
    return nc


_NC_CACHE = None
LAST_RESULT = None


def _get_graph():
    global _NC_CACHE
    if _NC_CACHE is None:
        _NC_CACHE = _build_graph()
    return _NC_CACHE


def _rot_matrix():
    # rot(q)[d] = -q[d+32] (d<32) ; q[d-32] (d>=32), per 64-block; 2 blocks.
    r64 = np.zeros((64, 64), dtype=np.float32)
    for d in range(32):
        r64[d, d + 32] = -1.0
        r64[d + 32, d] = 1.0
    r = np.zeros((128, 128), dtype=np.float32)
    r[0:64, 0:64] = r64
    r[64:128, 64:128] = r64
    return r


def _maskpat(core):
    """Packed [128, 1536] multiplicative window mask for one core.

    Column b*512 + c + i*128 + u corresponds to key row ki of k-tile kt
    against query column (qlo//128 + i)*128 + u; valid iff the key is in
    the causal 256-window and (core 0) not a zero-padded halo row.
    """
    pat = np.zeros((128, 1536), dtype=np.float32)
    ki = np.arange(128)[:, None]
    u = np.arange(128)[None, :]
    for (b, c, kt, qlo, qhi) in SCORE_BLOCKS:
        for i in range((qhi - qlo) // 128):
            qj = qlo + i * 128 + u
            k_pad = kt * 128 + ki
            valid = (qj <= k_pad) & (k_pad <= qj + HALO)
            if core == 0:
                valid = valid & (k_pad >= HALO)
            pat[:, b * 512 + c + i * 128 : b * 512 + c + (i + 1) * 128] = valid
    return pat.astype(ml_dtypes.bfloat16)


def kernel(x, mask, cos, sin, Wq, bq, Wk, Wv, bv, Wo, bo):
    x = np.asarray(x, dtype=np.float32)
    cos = np.asarray(cos, dtype=np.float32)
    sin = np.asarray(sin, dtype=np.float32)
    B = x.shape[0]
    assert (B, S, E) == x.shape

    bf = lambda a: np.ascontiguousarray(a).astype(ml_dtypes.bfloat16)
    wq_b, wk_b, wv_b, wo_b = bf(Wq), bf(Wk), bf(Wv), bf(Wo)
    rt_b = bf(_rot_matrix().T)
    bq_t = np.ascontiguousarray(
        np.asarray(bq, np.float32).reshape(NET, 128).T)
    bo_t = np.ascontiguousarray(
        np.asarray(bo, np.float32).reshape(NET, 128).T)
    bv_t = np.ascontiguousarray(
        np.tile(np.asarray(bv, np.float32)[None, :], (128, 1)))

    in_maps = []
    for c in range(NCORES):
        lo = c * SL - HALO
        xp = np.zeros((SK, E), dtype=np.float32)
        cp = np.zeros((SK, D), dtype=np.float32)
        sp = np.zeros((SK, D), dtype=np.float32)
        src_lo = max(lo, 0)
        dst_lo = src_lo - lo
        xp[dst_lo:] = x[0, src_lo : lo + SK]
        cp[dst_lo:] = cos[0, src_lo : lo + SK]
        sp[dst_lo:] = sin[0, src_lo : lo + SK]
        in_maps.append({
            "xT": bf(xp.T),
            "wq": wq_b, "wk": wk_b, "wv": wv_b, "wo": wo_b,
            "rt": rt_b,
            "bq": bq_t, "bo": bo_t, "bv": bv_t,
            "cosT": bf(np.tile(cp.T, (2, 1))),
            "sinT": bf(np.tile(sp.T, (2, 1))),
            "maskpat": _maskpat(c),
        })

    nc = _get_graph()
    trace = bool(os.environ.get("BASS_KERNEL_TRACE"))
    if trace:
        _ensure_ntff_hook()
    res = run_bass_kernel_spmd(
        nc, in_maps, core_ids=list(range(NCORES)), trace=trace
    )
    global LAST_RESULT
    LAST_RESULT = res

    out = np.empty((1, S, E), dtype=np.float32)
    for c in range(NCORES):
        out[0, c * SL : (c + 1) * SL, :] = (
            res.results[c]["out"].astype(np.float32).T)
    return out


if __name__ == "__main__":
    import reference
    inputs = reference.setup_inputs()
    inputs = {k: np.asarray(v) for k, v in inputs.items()}
    got = kernel(**inputs)
    exp = np.asarray(reference.reference(**inputs))
    err = np.abs(got - exp).max() / np.abs(exp).max()
    print("rel err:", err)

